# revision 9
# baseline (speedup 1.0000x reference)
"""AsyncCrossModalConsistencyLoss distributed Bass kernel for 8 TRN2 NeuronCores.

Data-parallel: batch dim (B=8) sharded one element per core. Each core:
  - loads its [4096, 512] visual/audio shard as f32 via HWDGE (nc.sync)
    1 MB chunks on a single queue (measured 469 GB/s sustained vs
    371 GB/s for the SWDGE cast-in-flight path), tail-tapered so the
    last chunks are 2/1/1 tiles to shorten the post-DMA critical path
  - per [128,512] tile: DVE cast f32->bf16, row sum-of-squares (ScalarE
    activation accum + DVE scalar_tensor_tensor accum), prod=v*a (DVE,
    bf16 2x), batched 1/max(norm,eps), then TensorE matmuls accumulate
    sum_s v_hat / sum_s a_hat and the sync dot-sum in PSUM
  - 4-op epilogue: total=<sumv,suma> (stt accum), sync=reduce(sync_ps),
    z=cA*total+cS*sync (stt accum against host-precomputed per-core
    constants), loss/8 = Relu(z + cM) via one ScalarE activation
AllReduce(add) over the 8 cores produces the global mean loss.

Host precompute (per core, from target w in {0,1}): sgn = 2w-1,
  cA = sgn*C_ASYNC/8, cS = -sgn*(C_SYNC+C_ASYNC)/8,
  cM = (0.1 + 0.9*w)*MARGIN/8
so that loss/8 = relu(cA*total + cS*sync + cM) exactly matches
  w*relu(async-sync+M) + (1-w)*relu(sync-async+0.1M), scaled by 1/8.
"""

import numpy as np

import concourse.bass as bass
import concourse.tile as tile
from concourse import bacc, mybir
from concourse.bass_utils import run_bass_kernel_spmd

N_CORES = 8
S = 4096
D = 512
P = 128
NT = S // P              # 32 compute tiles of [128, 512]
FREE = NT * D            # 16384 columns per partition

# tiles per DMA/compute chunk; 1 MB bulk chunks, tapered tail
PLAN = (4, 4, 4, 4, 4, 4, 4, 2, 1, 1)
assert sum(PLAN) == NT

EPS_DIV = 1e-8
MARGIN = 0.5
C_SYNC = 1.0 / S
C_ASYNC = 1.0 / (S * (S - 1) + EPS_DIV)

F32 = mybir.dt.float32
BF16 = mybir.dt.bfloat16
AF = mybir.ActivationFunctionType
OP = mybir.AluOpType


def _build(collective=True, reps=1, dma_mode="hwdge_f32"):
    """reps>1: wrap the body in tc.For_i for differential timing (no
    collective in that mode — collectives can't sit in control flow)."""
    import contextlib

    nc = bacc.Bacc(
        "TRN2", target_bir_lowering=False, debug=False,
        num_devices=N_CORES if collective else 1,
    )
    v_ext = nc.dram_tensor("v", [S, D], F32, kind="ExternalInput")
    a_ext = nc.dram_tensor("a", [S, D], F32, kind="ExternalInput")
    c_ext = nc.dram_tensor("c", [1, 3], F32, kind="ExternalInput")
    out_ext = nc.dram_tensor("out", [1, 1], F32, kind="ExternalOutput")

    # Row s = p*NT + n lands on partition p, tile n: contiguous 64KB per
    # partition in DRAM -> ideal DMA pattern. Any row->(p,n) bijection works
    # because every reduction here is symmetric over rows.
    v_re = v_ext.ap().rearrange("(p n) d -> p (n d)", p=P)
    a_re = a_ext.ap().rearrange("(p n) d -> p (n d)", p=P)

    with tile.TileContext(nc) as tc:
        with (
            tc.tile_pool(name="big", bufs=1) as big,
            tc.tile_pool(name="scratch", bufs=2) as scratch,
            tc.tile_pool(name="small", bufs=3) as small,
            tc.tile_pool(name="psum", bufs=1, space="PSUM") as psum,
            tc.tile_pool(name="dram", bufs=1, space="DRAM") as dram,
        ):
            sb_dt = F32 if dma_mode == "hwdge_f32" else BF16
            v_sb = big.tile([P, FREE], sb_dt)
            a_sb = big.tile([P, FREE], sb_dt)
            c_sb = big.tile([1, 3], F32)
            eps_b = big.tile([P, 1], F32)
            nc.vector.memset(eps_b[:], 1e-24)
            nc.sync.dma_start(c_sb[:], c_ext[:])
            loop_cm = tc.For_i(0, reps) if reps > 1 else contextlib.nullcontext()
            with loop_cm:
                _body(nc, tc, scratch, small, psum, v_sb, a_sb, c_sb, eps_b,
                      v_re, a_re, dma_mode)
            lscaled = _EPILOGUE_OUT[0]

            if collective:
                loss_bounce = dram.tile([1, 1], F32)
                out_bounce = dram.tile([1, 1], F32)
                nc.sync.dma_start(loss_bounce[:], lscaled[:])
                nc.gpsimd.collective_compute(
                    "AllReduce",
                    OP.add,
                    replica_groups=[list(range(N_CORES))],
                    ins=[loss_bounce.opt()],
                    outs=[out_bounce.opt()],
                )
                nc.sync.dma_start(out_ext[:], out_bounce[:])
            else:
                nc.sync.dma_start(out_ext[:], lscaled[:])

    nc.compile()
    return nc


_EPILOGUE_OUT = [None]


def _body(nc, tc, scratch, small, psum, v_sb, a_sb, c_sb, eps_b,
          v_re, a_re, dma_mode):
    cast = dma_mode == "hwdge_f32"
    # All input DMAs on the single SP HWDGE queue (fastest measured), v
    # then a per chunk so the v tiles land first and their casts/squares
    # overlap the a drain.
    col = 0
    for tpc in PLAN:
        sl = slice(col * D, (col + tpc) * D)
        if cast:
            nc.sync.dma_start(v_sb[:, sl], v_re[:, sl])
            nc.sync.dma_start(a_sb[:, sl], a_re[:, sl])
        else:
            nc.gpsimd.dma_start(v_sb[:, sl], v_re[:, sl])
            nc.gpsimd.dma_start(a_sb[:, sl], a_re[:, sl])
        col += tpc

    sumv_ps = psum.tile([1, D], F32)
    suma_ps = psum.tile([1, D], F32)
    sync_ps = psum.tile([1, D], F32)

    t0 = 0
    for ci, tpc in enumerate(PLAN):
        first = ci == 0
        last = ci == len(PLAN) - 1
        # ss: cols [0:tpc] = sum v^2 per tile, [tpc:2*tpc] = sum a^2
        ss = small.tile([P, 2 * tpc], F32, tag=f"ss{tpc}")
        vbs, abs_, prods = [], [], []
        for j in range(tpc):
            t = t0 + j
            sl = slice(t * D, (t + 1) * D)
            if cast:
                vb_t = scratch.tile([P, D], BF16, tag=f"vb{j}")
                nc.vector.tensor_copy(vb_t[:], v_sb[:, sl])
                vb = vb_t[:]
                ab_t = scratch.tile([P, D], BF16, tag=f"ab{j}")
                nc.vector.tensor_copy(ab_t[:], a_sb[:, sl])
                ab = ab_t[:]
            else:
                vb = v_sb[:, sl]
                ab = a_sb[:, sl]
            vbs.append(vb)
            abs_.append(ab)
            # square outputs are junk (only the accums are used); one
            # shared tile per engine — writes are engine-serial anyway
            sq_v = scratch.tile([P, D], BF16, tag="sqj_act")
            nc.scalar.activation(
                sq_v[:], vb, AF.Square, accum_out=ss[:, j:j + 1],
            )
            sq_a = scratch.tile([P, D], BF16,
                                tag="sqj_act" if j < 1 else "sqj_dve")
            if j < 1:
                # ScalarE takes the first a-square of each chunk, DVE the
                # rest via scalar_tensor_tensor accum
                # (InstTensorTensorReduce faults on this HW)
                nc.scalar.activation(
                    sq_a[:], ab, AF.Square,
                    accum_out=ss[:, tpc + j:tpc + j + 1],
                )
            else:
                nc.vector.scalar_tensor_tensor(
                    out=sq_a[:], in0=ab, scalar=1.0, in1=ab,
                    op0=OP.mult, op1=OP.mult,
                    accum_out=ss[:, tpc + j:tpc + j + 1],
                )
            # prod = v*a (bf16 2x mode); its weighted row-sum goes
            # through the PE below, so no per-row dot accum is needed
            prod = scratch.tile([P, D], BF16, tag=f"prod{j}")
            nc.vector.tensor_tensor(
                out=prod[:], in0=vb, in1=ab, op=OP.mult,
            )
            prods.append(prod)

        # Batched 1/max(norm, eps) for the whole chunk. The sqrt bias
        # keeps sqrt(0) finite, matching F.normalize's max(norm, 1e-12)
        # for all realizable inputs.
        nrm = small.tile([P, 2 * tpc], F32, tag=f"nrm{tpc}")
        nc.scalar.activation(nrm[:], ss[:], AF.Sqrt, bias=eps_b[:])
        inv = small.tile([P, 2 * tpc], F32, tag=f"inv{tpc}")
        nc.vector.reciprocal(inv[:], nrm[:])
        inv_b = small.tile([P, 2 * tpc], BF16, tag=f"invb{tpc}")
        nc.vector.tensor_copy(inv_b[:], inv[:])
        invva_b = small.tile([P, tpc], BF16, tag=f"invva{tpc}")
        nc.vector.tensor_mul(invva_b[:], inv[:, 0:tpc], inv[:, tpc:])

        for j in range(tpc):
            st = first and j == 0
            sp = last and j == tpc - 1
            nc.tensor.matmul(
                sumv_ps[:], lhsT=inv_b[:, j:j + 1], rhs=vbs[j],
                start=st, stop=sp,
            )
            nc.tensor.matmul(
                suma_ps[:], lhsT=inv_b[:, tpc + j:tpc + j + 1], rhs=abs_[j],
                start=st, stop=sp,
            )
            # sync row: [1,D] += invva.T @ (v*a); summed in epilogue
            nc.tensor.matmul(
                sync_ps[:], lhsT=invva_b[:, j:j + 1], rhs=prods[j][:],
                start=st, stop=sp,
            )
        t0 += tpc

    # ---- epilogue: 5 ops on partition 0 ----
    # t2 = [total, sync]; z = cA*total + cS*sync; out = relu(z + cM)
    suma_sb = small.tile([1, D], F32)
    nc.scalar.copy(suma_sb[:], suma_ps[:])
    t2 = small.tile([1, 2], F32)
    junk1 = scratch.tile([P, D], F32, tag="junk1")
    nc.vector.scalar_tensor_tensor(
        out=junk1[0:1, :], in0=sumv_ps[:], scalar=1.0, in1=suma_sb[:],
        op0=OP.mult, op1=OP.mult, accum_out=t2[:, 0:1],
    )
    nc.vector.tensor_reduce(
        out=t2[:, 1:2], in_=sync_ps[:], op=OP.add,
        axis=mybir.AxisListType.X,
    )
    junk2 = small.tile([1, 2], F32)
    z = small.tile([1, 1], F32)
    nc.vector.scalar_tensor_tensor(
        out=junk2[:], in0=t2[:], scalar=1.0, in1=c_sb[:, 0:2],
        op0=OP.mult, op1=OP.mult, accum_out=z[:],
    )
    lscaled = small.tile([1, 1], F32)
    nc.scalar.activation(lscaled[:], z[:], AF.Relu, bias=c_sb[:, 2:3])
    _EPILOGUE_OUT[0] = lscaled


_NC = None


def _get_nc():
    global _NC
    if _NC is None:
        _NC = _build()
    return _NC


def make_in_maps(visual_features, audio_features, targets):
    vf = np.asarray(visual_features)
    af = np.asarray(audio_features)
    tg = np.asarray(targets)
    maps = []
    for i in range(N_CORES):
        w = float(tg[i])
        sgn = 2.0 * w - 1.0
        cA = sgn * C_ASYNC / N_CORES
        cS = -sgn * (C_SYNC + C_ASYNC) / N_CORES
        cM = (0.1 + 0.9 * w) * MARGIN / N_CORES
        maps.append(
            {
                "v": np.ascontiguousarray(vf[i], dtype=np.float32),
                "a": np.ascontiguousarray(af[i], dtype=np.float32),
                "c": np.array([[cA, cS, cM]], dtype=np.float32),
            }
        )
    return maps


def kernel(visual_features, audio_features, targets):
    nc = _get_nc()
    in_maps = make_in_maps(visual_features, audio_features, targets)
    res = run_bass_kernel_spmd(nc, in_maps, core_ids=list(range(N_CORES)))
    out = np.asarray(res.results[0]["out"], dtype=np.float32)
    return out.reshape(())


if __name__ == "__main__":
    rng = np.random.default_rng(0)
    v = rng.standard_normal((N_CORES, S, D)).astype(np.float32)
    a = rng.standard_normal((N_CORES, S, D)).astype(np.float32)
    t = rng.integers(0, 2, (N_CORES,)).astype(np.int32)
    print(kernel(visual_features=v, audio_features=a, targets=t))
